# revision 95
# baseline (speedup 1.0000x reference)
"""Associative-embedding loss on 8 Trainium2 NeuronCores.

Data-parallel over batch N=32: each of the 8 cores handles 4 images.

Strategy (v3): batched row-gather + per-image pipelined reduction.
  The per-element tag gather is done with 4 InstDMAGatherAnt ops (one per
  image; the int16 row-index range caps one gather at ~32k rows of 256B,
  i.e. one image at 64-f32 granularity). Each gather pulls 544 rows of
  64 f32 (the 256B row containing each joint's tag value) into SBUF at
  slot (p = 32*(k%4) + m, j = k//4). A host-precomputed one-hot *
  visibility mask extracts the wanted element from each row:
      gm = rows * mask      (DVE)
      S1part[p] = sum(gm)   (DVE reduce)        = sum_k g*vis   (k-partial)
      S2part[p] = sum(gm^2) (ACT Square+accum)  = sum_k g^2*vis
  Everything downstream runs PER IMAGE, pipelined against the remaining
  gathers (image ni's gather holds all of its data):
    - PE merge matmuls against a host 0/1 matrix M32[p,mq]=[p%32==mq]
      give per-person S1 both as a column block and as a row block
      (the row form kills the PE transpose on the critical path).
    - pull_i = sum_m segp*(S2/cnt - mean^2): the S2/cnt part folds into a
      single matmul with a host column (segp*rc merged through M), the
      mean^2 part reuses the exp bias column; constant offsets ride on a
      host-only bias matmul.
    - push: a [32,32] pairwise block per image via 2 rank-1 matmuls
      (m_p*m_q + (-m_q^2/2 + (BIG/2)v_q)), then exp(2*x + bias_p) on ACT
      with bias_p = -m_p^2 - BIG and fused row sums. Invalid columns are
      killed by the BIG terms; cross-image pairs don't exist by layout.
    - per-image scaling (1/max(nt,1), 0.5*gate/max(nt(nt-1),1), validity)
      is folded into host segment columns, so single-column matmuls
      accumulate the final [4,2] (pull_i, push_i) directly in PSUM.
  Host-side preprocessing touches only the joints tensor (index and
  visibility arithmetic) plus constant matrices; all tag data is read and
  reduced on device. Host concatenates the 8 x [4,2] outputs and means.
"""

import numpy as np
import ml_dtypes
from contextlib import ExitStack

import concourse.bass as bass
import concourse.tile as tile
from concourse import mybir, library_config
from concourse.bass_utils import run_bass_kernel_spmd

# Problem constants (hardcoded per contract).
N, K, H, W, M = 32, 17, 256, 256, 30
NCORES = 8
NLOC = N // NCORES          # images per core
KHW = K * H * W             # 1114112 flat tag elements per image
MP = 32                     # padded persons per image
P = NLOC * MP               # 128 partitions; home slots q = 32*ni + m
BIG = 30.0                  # exp(-BIG) ~ 9e-14 kills masked pairs

EW = 64                     # gathered row width (f32) = 256B descriptor
ROWS = KHW // EW            # 17408 rows per image (< 32767 int16 limit)
NIDX = 544                  # indices per gather = 32 * 17
JC = 5                      # gather out columns = ceil(544/128)
GW = JC * EW                # 320 floats per partition per tile
IDXC = NIDX // 16           # 34 int16 idx columns per gather

# mask ships separately as bf16 (values are exactly 0/1): [128, 4*GW]
# aux (f32, [128, AUXW]) column layout
A_SEGP4 = 0                 # ..+4: valid * pull_scale, bucketed per image
A_SEGQ4 = A_SEGP4 + NLOC    # ..+4: valid * push_scale, bucketed per image
A_C2 = A_SEGQ4 + NLOC       # ..+4: pullA columns (segp*rc merged via M)
A_SC2 = A_C2 + NLOC         # 2.0 (exp scale)
A_ONE = A_SC2 + 1           # 1.0 (offset-mm lhsT)
A_OFF4 = A_ONE + 1          # ..+4: push offsets spread over image blocks
A_M32 = A_OFF4 + NLOC       # ..+32: M32[p, mq] = [p%32 == mq]
A_M32C = A_M32 + MP         # ..+128: per-image rc-folded M32 (mean merge)
AUXW = A_M32C + NLOC * MP

# rows (f32, [2, RW]): 128-wide rows at partitions 0-1
R_ONES = 0                  # ones in BOTH rows (K=2 matmul lhsT)
R_RCT = P                   # row0: rc as a row
R_AT = 2 * P                # row0: aT (device-written); row1: (BIG/2)v - BIG/2
R_OFF = 3 * P               # ..+4 row0: per-image push constant offsets
RW = 3 * P + NLOC

f32 = mybir.dt.float32
i16 = mybir.dt.int16
Alu = mybir.AluOpType
Act = mybir.ActivationFunctionType


# scheduling-shape knobs (tuned empirically against the cost model)
VARIANT = {
    "mean_in_l1": True,     # emit nm2/meanT/aT inside the extraction loop
    "split_mask": False,    # per-image mask DMAs vs one aux DMA
    "split_idx": True,      # separate first-gather idx DMA
}


def build_nc(debug: bool = False) -> bass.Bass:
    nc = bass.Bass()
    tags_d = nc.declare_dram_parameter("tags", [NLOC, KHW], f32, isOutput=False)
    idx_d = nc.declare_dram_parameter("idx", [P, NLOC * IDXC], i16, isOutput=False)
    aux_d = nc.declare_dram_parameter("aux", [P, AUXW], f32, isOutput=False)
    mask_d = nc.declare_dram_parameter("maskb", [P, NLOC * GW], mybir.dt.bfloat16,
                                       isOutput=False)
    rows_d = nc.declare_dram_parameter("rows", [2, RW], f32, isOutput=False)
    out_d = nc.declare_dram_parameter("out", [1, 2 * NLOC], f32, isOutput=True)

    with tile.TileContext(nc) as tc:
        with ExitStack() as ctx:
            _body(ctx, tc, nc, tags_d, idx_d[:], aux_d[:], mask_d[:],
                  rows_d[:], out_d[:])
    # raw Bass skips Bacc's extended-inst codegen pass; without it walrus
    # sees empty .instr on the library-reload ISA op ("ISA wrong length")
    mybir.codegen_inst_isa_subclasses(nc)
    _split_multi_waits(nc, max_waits=1)
    return nc


def _split_multi_waits(nc, max_waits=1):
    """Walrus codegen rejects instructions with too many sync-wait commands.
    Split excess waits onto same-engine nops placed before the instruction."""
    import bass_rust
    fn = nc.m.functions[0]
    for bb in fn.blocks:
        changed = True
        while changed:
            changed = False
            for inst in list(bb.instructions):
                si = inst.sync_info
                if si is None or not si.on_wait or len(si.on_wait) <= max_waits:
                    continue
                waits = list(si.on_wait)
                keep, rest = waits[:max_waits], waits[max_waits:]
                nops = []
                for i in range(0, len(rest), max_waits):
                    nop_inst = nc.engines[inst.engine].nop().ins
                    nop_inst.sync_info = bass_rust.SyncInfo(
                        on_wait=rest[i:i + max_waits], on_update=[])
                    nops.append(nop_inst)
                inst.sync_info = bass_rust.SyncInfo(
                    on_wait=keep, on_update=list(si.on_update))
                for b2 in fn.blocks:
                    lst = b2.instructions
                    for i in range(len(lst) - 1, -1, -1):
                        if any(lst[i].name == n.name for n in nops):
                            del lst[i]
                idx = next(i for i, x in enumerate(bb.instructions)
                           if x.name == inst.name)
                for j, n in enumerate(nops):
                    bb.instructions.insert(idx + j, n)
                changed = True
                break


def _body(ctx, tc, nc, tags_d, idx, aux, mask, rows, out):
    pool = ctx.enter_context(tc.tile_pool(name="main", bufs=1))
    psum = ctx.enter_context(tc.tile_pool(name="psum", bufs=1, space="PSUM"))

    # ---- input DMAs (SP HWDGE queue; idx first — gathers gate on it).
    # Gather 0's indices go in a minimal first DMA so its desc-gen starts
    # as early as possible; the mask is split per image so its transfers
    # don't block gather transfers on the DMA engines.
    idxs = pool.tile([P, NLOC * IDXC], i16)
    if VARIANT["split_idx"]:
        nc.sync.dma_start(out=idxs[:, 0:IDXC], in_=idx[:, 0:IDXC])
        nc.sync.dma_start(out=idxs[:, IDXC:], in_=idx[:, IDXC:])
    else:
        nc.sync.dma_start(out=idxs, in_=idx)
    auxt = pool.tile([P, AUXW], f32)
    nc.sync.dma_start(out=auxt, in_=aux)
    maskt = pool.tile([P, NLOC * GW], mybir.dt.bfloat16)
    nc.sync.dma_start(out=maskt, in_=mask)
    rowt = pool.tile([2, RW], f32)
    nc.sync.dma_start(out=rowt, in_=rows)

    M32 = auxt[:, A_M32:A_M32 + MP]

    nc.gpsimd.load_library(library_config.mlp)

    # ---- PSUM accumulators --------------------------------------------------
    # Each padded to a full 2KB bank row: PSUM accumulation-group tracking is
    # per 2KB "zero region", and ps_push's group stays pending for a long
    # stretch — sharing its bank would break other tiles' groups.
    BANKC = 512                                       # f32 per 2KB bank row
    psS1_b = psum.tile([P, BANKC], f32, tag="psS1", name="psS1_b")
    psrow_b = psum.tile([1, BANKC], f32, tag="psrow", name="psrow_b")
    pspush_b = psum.tile([P, BANKC], f32, tag="pspush", name="pspush_b")
    pspush2_b = psum.tile([P, BANKC], f32, tag="pspush2", name="pspush2_b")
    psF_b = psum.tile([1, BANKC], f32, tag="psF", name="psF_b")
    psG_b = psum.tile([1, BANKC], f32, tag="psG", name="psG_b")
    psS1 = psS1_b[:, 0:1]
    ps_row = psrow_b[0:1, 0:P]
    # alternating pairwise-block banks: PSUM dep tracking is tile-coarse, so
    # one shared tile serializes image i's mm2 behind image i-1's exp read
    ps_pushes = [pspush_b[:, 0:MP], pspush2_b[:, 0:MP]]
    # per-image output accumulators: psF cell i = pullA_i (start) + pullB_i
    # (stop); psG cell i = push offset (start, early) + push_i (stop)
    psF = psF_b[0:1, 0:NLOC]
    psG = psG_b[0:1, 0:NLOC]

    # ---- gathers: 4 x 544 rows of 64 f32 ------------------------------------
    gts = []
    for ni in range(NLOC):
        gt = pool.tile([P, JC, EW], f32, tag=f"gt{ni}")
        # only slots i>=544 (in the last column) are never written by the
        # gather; zero the column first (the gather overwrites its slots)
        nc.vector.memset(gt[:, JC - 1, :], 0.0)
        gts.append(gt)
    for ni in range(NLOC):
        nc.gpsimd.dma_gather(
            out_ap=gts[ni][:],
            in_ap=tags_d[ni].rearrange("(r e) -> r e", e=EW),
            idxs_ap=idxs[:, ni * IDXC:(ni + 1) * IDXC],
            num_idxs=NIDX,
            num_idxs_reg=NIDX,
            elem_size=EW,
        )

    # ---- per-image pipelined extraction + stats -----------------------------
    xS1 = pool.tile([P, NLOC], f32)                   # DVE-written S1 partials
    xS2 = pool.tile([P, NLOC], f32)                   # ACT-written S2 partials
    sq = pool.tile([P, GW], f32)                      # ACT scratch
    meanT = pool.tile([1, P], f32)
    nm2 = pool.tile([P, 1], f32)
    rowsum = pool.tile([P, 1], f32)
    pexp = pool.tile([P, MP], f32)

    # constant push offsets open the psG accumulation group (host operands)
    nc.tensor.matmul(out=psG[:], lhsT=auxt[:, A_ONE:A_ONE + 1],
                     rhs=auxt[:, A_OFF4:A_OFF4 + NLOC],
                     tile_position=(0, 0), start=True, stop=False)

    def mean_prep(ni):
        sl = slice(ni * MP, (ni + 1) * MP)
        cs = slice(ni * MP, (ni + 1) * MP)
        # nm2 = -mean^2; psS1 appears once as tensor, once as a (per-
        # partition) scalar operand — walrus allows one PSUM tensor input
        nc.vector.tensor_scalar(out=nm2[sl], in0=psS1[sl, 0:1],
                                scalar1=psS1[sl, 0:1], scalar2=-1.0,
                                op0=Alu.mult, op1=Alu.mult)
        nc.vector.tensor_tensor(out=meanT[0:1, cs], in0=ps_row[0:1, cs],
                                in1=rowt[0:1, R_RCT + ni * MP:R_RCT + (ni + 1) * MP],
                                op=Alu.mult)
        nc.vector.scalar_tensor_tensor(out=rowt[0:1, R_AT + ni * MP:R_AT + (ni + 1) * MP],
                                       in0=meanT[0:1, cs], scalar=-0.5,
                                       in1=meanT[0:1, cs],
                                       op0=Alu.mult, op1=Alu.mult)

    # loop 1: extraction + merges per image (pipelined against the gathers)
    for ni in range(NLOC):
        sl = slice(ni * MP, (ni + 1) * MP)
        cs = slice(ni * MP, (ni + 1) * MP)           # row-column slice
        gflat = gts[ni][:].rearrange("p a b -> p (a b)")
        gm = pool.tile([P, GW], f32, tag=f"gm{ni}")
        nc.vector.tensor_tensor(out=gm, in0=gflat,
                                in1=maskt[:, ni * GW:(ni + 1) * GW], op=Alu.mult)
        nc.vector.reduce_sum(out=xS1[:, ni:ni + 1], in_=gm,
                             axis=mybir.AxisListType.X)
        if ni < NLOC - 1:
            # S2 on ACT for the early images
            nc.scalar.activation(out=sq, in_=gm, func=Act.Square,
                                 accum_out=xS2[:, ni:ni + 1])

        # per-person mean (rc-folded merge) and S1 row block
        nc.tensor.matmul(out=psS1[sl, 0:1],
                         lhsT=auxt[:, A_M32C + ni * MP:A_M32C + (ni + 1) * MP],
                         rhs=xS1[:, ni:ni + 1],
                         tile_position=(0, ni * MP), start=True, stop=True)
        nc.tensor.matmul(out=ps_row[0:1, cs], lhsT=xS1[:, ni:ni + 1],
                         rhs=M32, tile_position=(0, 0), start=True, stop=True)
        if VARIANT["mean_in_l1"]:
            mean_prep(ni)
        if ni == NLOC - 1:
            # last image's S2 on DVE (ACT is saturated here): first half
            # emitted now so the greedy scheduler can slot it into DVE idle
            # gaps while the mean chain waits on PE merges
            sq3 = pool.tile([P, GW], f32)
            nc.vector.tensor_tensor(out=sq3[:, 0:GW // 2],
                                    in0=gm[:, 0:GW // 2],
                                    in1=gm[:, 0:GW // 2], op=Alu.mult)
            gm3 = gm
            continue
        # pullA (opens cell i) then pullB (closes it): pull_i complete in psF
        nc.tensor.matmul(out=psF[0:1, ni:ni + 1],
                         lhsT=auxt[:, A_C2 + ni:A_C2 + ni + 1],
                         rhs=xS2[:, ni:ni + 1],
                         tile_position=(0, 0), start=True, stop=False)
        nc.tensor.matmul(out=psF[0:1, ni:ni + 1],
                         lhsT=auxt[sl, A_SEGP4 + ni:A_SEGP4 + ni + 1],
                         rhs=nm2[sl], tile_position=(ni * MP, 0),
                         start=False, stop=True)

    # loop 2: per-image pairwise block, exp, output matmuls
    for ni in range(NLOC):
        sl = slice(ni * MP, (ni + 1) * MP)
        cs = slice(ni * MP, (ni + 1) * MP)
        if not VARIANT["mean_in_l1"]:
            mean_prep(ni)
        # m_p*m_q, then a K=2 broadcast of (aT_q + hv_q) in one matmul
        ps_push = ps_pushes[ni % 2]
        nc.tensor.matmul(out=ps_push[sl, :], lhsT=meanT[0:1, cs],
                         rhs=meanT[0:1, cs], tile_position=(0, ni * MP),
                         start=True, stop=False)
        nc.tensor.matmul(out=ps_push[sl, :], lhsT=rowt[0:2, R_ONES:R_ONES + MP],
                         rhs=rowt[0:2, R_AT + ni * MP:R_AT + (ni + 1) * MP],
                         tile_position=(0, ni * MP), start=False, stop=True)
        nc.scalar.activation(out=pexp[sl, :], in_=ps_push[sl, :], func=Act.Exp,
                             scale=auxt[sl, A_SC2:A_SC2 + 1],
                             bias=nm2[sl, 0:1], accum_out=rowsum[sl, 0:1])
        nc.tensor.matmul(out=psG[0:1, ni:ni + 1],
                         lhsT=auxt[sl, A_SEGQ4 + ni:A_SEGQ4 + ni + 1],
                         rhs=rowsum[sl, 0:1], tile_position=(ni * MP, 0),
                         start=False, stop=(ni == NLOC - 1))

    # image 3's pull tail: first S2 half reduces early (gap-filler), the
    # second half is gated behind aT so the exp chain keeps DVE priority;
    # both partial reductions accumulate into the pull cell via two matmuls
    ni = NLOC - 1
    sl = slice(ni * MP, (ni + 1) * MP)
    xtra = pool.tile([P, 2], f32)
    nc.vector.reduce_sum(out=xtra[:, 0:1], in_=sq3[:, 0:GW // 2],
                         axis=mybir.AxisListType.X)
    nc.vector.tensor_scalar(out=sq3[0:1, GW // 2:GW // 2 + 1],
                            in0=rowt[0:1, R_AT + ni * MP:R_AT + ni * MP + 1],
                            scalar1=0.0, scalar2=None, op0=Alu.mult)
    nc.vector.tensor_tensor(out=sq3[:, GW // 2:GW], in0=gm3[:, GW // 2:GW],
                            in1=gm3[:, GW // 2:GW], op=Alu.mult)
    nc.vector.reduce_sum(out=xtra[:, 1:2], in_=sq3[:, GW // 2:GW],
                         axis=mybir.AxisListType.X)
    nc.tensor.matmul(out=psF[0:1, ni:ni + 1],
                     lhsT=auxt[:, A_C2 + ni:A_C2 + ni + 1],
                     rhs=xtra[:, 0:1],
                     tile_position=(0, 0), start=True, stop=False)
    nc.tensor.matmul(out=psF[0:1, ni:ni + 1],
                     lhsT=auxt[:, A_C2 + ni:A_C2 + ni + 1],
                     rhs=xtra[:, 1:2],
                     tile_position=(0, 0), start=False, stop=False)
    nc.tensor.matmul(out=psF[0:1, ni:ni + 1],
                     lhsT=auxt[sl, A_SEGP4 + ni:A_SEGP4 + ni + 1],
                     rhs=nm2[sl], tile_position=(ni * MP, 0),
                     start=False, stop=True)

    # f42 = [pull_0..3 | push_0..3]; each copy has a single PSUM input
    f42 = pool.tile([1, 2 * NLOC], f32)
    nc.vector.tensor_copy(out=f42[0:1, 0:NLOC], in_=psF)
    nc.vector.tensor_copy(out=f42[0:1, NLOC:2 * NLOC], in_=psG)
    nc.sync.dma_start(out=out, in_=f42)


_NC_CACHE = None


def _get_nc():
    global _NC_CACHE
    if _NC_CACHE is None:
        _NC_CACHE = build_nc()
    return _NC_CACHE


def make_in_maps(tags: np.ndarray, joints: np.ndarray):
    tags = np.ascontiguousarray(np.asarray(tags, dtype=np.float32))
    jt = np.asarray(joints)
    loc = np.clip(jt[..., 0], 0, KHW - 1).astype(np.int64)   # [N, M, K]
    vis = (jt[..., 1] > 0)                                   # [N, M, K]
    row = (loc // EW).astype(np.int16)                       # [N, M, K]
    off = (loc % EW).astype(np.int64)                        # [N, M, K]

    cnt = vis.sum(-1).astype(np.float64)                     # [N, M]
    person_valid = cnt > 0
    rc_nm = 1.0 / np.maximum(cnt, 1.0)                       # [N, M]
    nt = person_valid.sum(-1).astype(np.float64)             # [N]
    npr = nt * (nt - 1.0)
    pull_scale = 1.0 / np.maximum(nt, 1.0)                   # [N]
    push_scale = np.where(npr > 0, 0.5 / np.maximum(npr, 1.0), 0.0)

    # slot mapping within one image's gather: joint (m, k) -> unwrapped idx
    # position i = (k//4)*128 + 32*(k%4) + m;  p = i%128, j = i//128
    ms, ks = np.meshgrid(np.arange(MP), np.arange(K), indexing="ij")  # [32,17]
    ii = (ks // 4) * P + 32 * (ks % 4) + ms                  # [32, 17]

    qn = np.arange(P)                                        # home slots
    qni, qm = qn // MP, qn % MP

    in_maps = []
    for c in range(NCORES):
        sl = slice(c * NLOC, (c + 1) * NLOC)
        lrow, loff = row[sl], off[sl]                        # [4, 30, 17]
        lvis = vis[sl]

        # per-home-slot (q = 32*ni + m) quantities, pad persons zeroed
        rc_pad = np.ones((NLOC, MP))
        rc_pad[:, :M] = rc_nm[sl]
        pv_pad = np.zeros((NLOC, MP))
        pv_pad[:, :M] = person_valid[sl]
        rcq = rc_pad[qni, qm]                                # [128]
        vq = pv_pad[qni, qm]                                 # [128]
        plsq = pull_scale[sl][qni]                           # [128]
        pshq = push_scale[sl][qni]                           # [128]
        ntq = nt[sl][qni]                                    # [128]
        segp = vq * plsq
        segq = vq * pshq

        idx16 = np.zeros((P, NLOC * IDXC), dtype=np.int16)
        auxa = np.zeros((P, AUXW), dtype=np.float32)
        maskb = np.zeros((P, NLOC * GW), dtype=ml_dtypes.bfloat16)
        for ni in range(NLOC):
            iflat = np.zeros(NIDX, dtype=np.int16)
            # persons 30,31 are padding: idx 0 (valid row), mask 0
            iflat[ii[:M].ravel()] = lrow[ni].ravel()
            wrapped = iflat.reshape(IDXC, 16).T               # [16, 34]
            idx16[:, ni * IDXC:(ni + 1) * IDXC] = np.tile(wrapped, (8, 1))
            # mask: slot (p = i%128, j = i//128) col = j*64 + off
            mtile = np.zeros((P, JC, EW), dtype=np.float32)
            pslot = (ii[:M] % P).ravel()
            jslot = (ii[:M] // P).ravel()
            mtile[pslot, jslot, loff[ni].ravel()] = \
                lvis[ni].ravel().astype(np.float32)
            maskb[:, ni * GW:(ni + 1) * GW] = \
                mtile.reshape(P, GW).astype(ml_dtypes.bfloat16)
        for i in range(NLOC):
            ind = (qni == i).astype(np.float32)
            auxa[:, A_SEGP4 + i] = segp * ind
            auxa[:, A_SEGQ4 + i] = segq * ind
        for i in range(NLOC):
            # pullA col: c2_i[p] = (segp*rc)[32*i + p%32]
            auxa[:, A_C2 + i] = (segp * rcq)[MP * i + qn % MP]
        auxa[:, A_SC2] = 2.0
        auxa[:, A_ONE] = 1.0
        auxa[:, A_OFF4:A_OFF4 + NLOC] = \
            (-push_scale[sl] * nt[sl] / P)[None, :]
        m32 = (qn[:, None] % MP == np.arange(MP)[None, :]).astype(np.float32)
        auxa[:, A_M32:A_M32 + MP] = m32
        for i in range(NLOC):
            # rc-folded merge: psS1 column comes out as the per-person mean
            auxa[:, A_M32C + i * MP:A_M32C + (i + 1) * MP] = \
                m32 * rcq[i * MP:(i + 1) * MP][None, :]

        rowsa = np.zeros((2, RW), dtype=np.float32)
        rowsa[:, R_ONES:R_ONES + P] = 1.0
        rowsa[0, R_RCT:R_RCT + P] = rcq
        rowsa[1, R_AT:R_AT + P] = (BIG / 2.0) * vq - BIG / 2.0
        lnt = nt[sl]
        rowsa[0, R_OFF:R_OFF + NLOC] = -push_scale[sl] * lnt

        in_maps.append({
            "tags": tags[sl].reshape(NLOC, KHW),
            "idx": idx16,
            "aux": auxa,
            "maskb": maskb,
            "rows": rowsa,
        })
    return in_maps


def kernel(tags: np.ndarray, joints: np.ndarray, _bench_results=None):
    nc = _get_nc()
    in_maps = make_in_maps(tags, joints)
    res = run_bass_kernel_spmd(nc, in_maps, core_ids=list(range(NCORES)))
    if _bench_results is not None:
        _bench_results.append(res)
    per_image = np.concatenate(
        [r["out"].reshape(2, NLOC).T for r in res.results], axis=0)  # [32, 2]
    pull_loss = np.float32(per_image[:, 0].mean(dtype=np.float64))
    push_loss = np.float32(per_image[:, 1].mean(dtype=np.float64))
    return pull_loss, push_loss
